# revision 16
# baseline (speedup 1.0000x reference)
"""MultiHeadCrossAttention kernel for 8 trn2 NeuronCores.

Reference computation (fp32, per batch b):
    q = Q[b] @ W_q.T ; k = K[b] @ W_k.T ; v = V[b] @ W_v.T      (heads on columns)
    per head h: S = (q_h @ k_h.T) / 8 ; E = exp(S); A = E / E.sum(-1)
    out[b] = concat_h(A @ v_h) @ W_o.T ; rows with mask==0 zeroed

Sharding: 8 cores = (batch b in {0,1}) x (head-group hg in {0..3}, 4 heads each).
Each core computes a partial output  out_part[b] = concat(heads hg) @ W_o[:, cols].T
and the host sums the 4 partials per batch.

Per-core kernel layout (all matmul operands bf16, fp32 PSUM accumulation):
  - xqT/xkT/xvT: host-transposed [1024(in), 2048(seq)] so the contraction dim
    (model dim) lands on SBUF partitions.
  - qT/kT stored as [128, 2, 2048]: partition = local-head-dim % 128. Head pair
    hp occupies chunk hp; even head rows 0:64, odd head rows 64:128 - so the two
    scores matmuls of a pair use disjoint PE row groups and run concurrently.
  - Scores are computed transposed, ST[kpos, q], per 128-kpos chunk j; exp runs
    on ScalarE over a 2-bank [128, 1024] PSUM region (even|odd head halves) with
    the 1/8 scale fused.
  - PV: acc[128, 512] += [ones | 0*63 | v_h].T @ E  accumulated over j. Row 0 is
    the softmax denominator; rows 64:128 the unnormalized output ([d, q]).
  - Normalize: copy acc to SBUF (frees the PSUM bank fast), approx reciprocal
    of row 0, DRAM bounce + partition-broadcast read, one tensor_mul into the
    W_o lhsT layout (bf16).
  - W_o: final[q,:] accumulated over the 256 local dims; the mask (per-q 0/1)
    is applied by the PSUM->SBUF tensor_scalar multiply before the output DMA.
"""

import numpy as np
import ml_dtypes

import concourse.bass as bass
import concourse.bacc as bacc
import concourse.mybir as mybir
import concourse.tile as tile
from contextlib import ExitStack

F32 = mybir.dt.float32
BF16 = mybir.dt.bfloat16
AF = mybir.ActivationFunctionType

B = 2
SEQ = 2048          # Sq == Sk
D = 1024            # model dim
DL = 256            # local head dims per core (4 heads x 64)
HL = 4              # local heads
DH = 64             # head dim
NCORES = 8

_PROGRAM = None


def build_program():
    nc = bacc.Bacc("TRN2", target_bir_lowering=False)

    xqT = nc.declare_dram_parameter("xqT", [D, SEQ], BF16, isOutput=False)
    xkT = nc.declare_dram_parameter("xkT", [D, SEQ], BF16, isOutput=False)
    xvT = nc.declare_dram_parameter("xvT", [D, SEQ], BF16, isOutput=False)
    wq = nc.declare_dram_parameter("wq", [D, DL], BF16, isOutput=False)
    wk = nc.declare_dram_parameter("wk", [D, DL], BF16, isOutput=False)
    wv = nc.declare_dram_parameter("wv", [D, DL], BF16, isOutput=False)
    wo = nc.declare_dram_parameter("wo", [DL, D], BF16, isOutput=False)
    maskf = nc.declare_dram_parameter("maskf", [128, SEQ // 128], F32, isOutput=False)
    out_part = nc.declare_dram_parameter("out_part", [SEQ, D], F32, isOutput=True)

    r_dram = nc.dram_tensor("r_bounce", [16, 512], F32)  # recip bounce rows


    with tile.TileContext(nc) as tc, ExitStack() as ctx:
        const = ctx.enter_context(tc.tile_pool(name="const", bufs=1))
        proj = ctx.enter_context(tc.tile_pool(name="proj", bufs=1))
        xpool = ctx.enter_context(tc.tile_pool(name="xpool", bufs=4))
        epool = ctx.enter_context(tc.tile_pool(name="epool", bufs=6))
        opool = ctx.enter_context(tc.tile_pool(name="opool", bufs=2))
        ospool = ctx.enter_context(tc.tile_pool(name="ospool", bufs=4))
        rpool = ctx.enter_context(tc.tile_pool(name="rpool", bufs=4))
        pp = ctx.enter_context(tc.tile_pool(name="pp", bufs=2, space="PSUM"))
        stp = ctx.enter_context(tc.tile_pool(name="stp", bufs=2, space="PSUM"))
        accp = ctx.enter_context(tc.tile_pool(name="accp", bufs=2, space="PSUM"))

        # ---------------- constants ----------------
        wq_sb = const.tile([128, 8, DL], BF16)
        wk_sb = const.tile([128, 8, DL], BF16)
        wv_sb = const.tile([128, 8, DL], BF16)
        wo_sb = const.tile([128, 2, D], BF16)
        mask_sb = const.tile([128, SEQ // 128], F32)
        ones_sb = const.tile([1, 64], F32)
        nc.vector.memset(ones_sb[:], 1.0)
        nc.scalar.dma_start(wq_sb[:], wq[:].rearrange("(a p) d -> p a d", p=128))
        nc.sync.dma_start(wk_sb[:], wk[:].rearrange("(a p) d -> p a d", p=128))
        nc.scalar.dma_start(wv_sb[:], wv[:].rearrange("(a p) d -> p a d", p=128))
        nc.scalar.dma_start(wo_sb[:], wo[:].rearrange("(a p) d -> p a d", p=128))
        nc.scalar.dma_start(mask_sb[:], maskf[:])

        kT0_sb = proj.tile([128, SEQ], BF16)
        kT1_sb = proj.tile([128, SEQ], BF16)
        kTs = (kT0_sb, kT1_sb)
        qTs = [
            [proj.tile([128, 1024], BF16, name=f"qT{dm}_{h}") for h in range(2)]
            for dm in range(2)
        ]
        vaugs = [
            proj.tile([128, HL, 128], BF16, name=f"vaug{j}") for j in range(16)
        ]
        for j in range(16):
            nc.vector.memset(vaugs[j][:], 0.0)
            nc.vector.memset(vaugs[j][:, :, 0:1], 1.0)

        # ---------------- projections ----------------
        # order: k fully, then q fully, then v (per-j vaug tiles let PV start
        # as soon as the first v chunks land)
        for w_sb, xT, which in ((wk_sb, xkT, "k"),):
            for half in range(2):
                x_t = xpool.tile([128, 8, 1024], BF16, tag="x")
                for quarter in range(2):
                    nc.sync.dma_start(
                        x_t[:, quarter * 4 : (quarter + 1) * 4, :],
                        xT[
                            quarter * 512 : (quarter + 1) * 512,
                            half * 1024 : (half + 1) * 1024,
                        ].rearrange("(a p) q -> p a q", p=128),
                    )
                for dm in range(2):
                    for qc in range(2):
                        ps = pp.tile([128, 512], F32, tag="pp")
                        for ki in range(8):
                            nc.tensor.matmul(
                                ps[:],
                                lhsT=w_sb[:, ki, dm * 128 : (dm + 1) * 128],
                                rhs=x_t[:, ki, qc * 512 : (qc + 1) * 512],
                                start=(ki == 0),
                                stop=(ki == 7),
                            )
                        col = half * 1024 + qc * 512
                        if which == "k":
                            nc.scalar.copy(
                                out=kTs[dm][:, col : col + 512], in_=ps[:]
                            )
                        else:
                            nc.scalar.copy(
                                out=qTs[dm][half][:, qc * 512 : (qc + 1) * 512],
                                in_=ps[:],
                            )
        for w_sb, xT, which in ((wq_sb, xqT, "q"),):
            for half in range(2):
                x_t = xpool.tile([128, 8, 1024], BF16, tag="x")
                for quarter in range(2):
                    nc.sync.dma_start(
                        x_t[:, quarter * 4 : (quarter + 1) * 4, :],
                        xT[
                            quarter * 512 : (quarter + 1) * 512,
                            half * 1024 : (half + 1) * 1024,
                        ].rearrange("(a p) q -> p a q", p=128),
                    )
                for dm in range(2):
                    for qc in range(2):
                        ps = pp.tile([128, 512], F32, tag="pp")
                        for ki in range(8):
                            nc.tensor.matmul(
                                ps[:],
                                lhsT=w_sb[:, ki, dm * 128 : (dm + 1) * 128],
                                rhs=x_t[:, ki, qc * 512 : (qc + 1) * 512],
                                start=(ki == 0),
                                stop=(ki == 7),
                            )
                        col = half * 1024 + qc * 512
                        if which == "k":
                            nc.scalar.copy(
                                out=kTs[dm][:, col : col + 512], in_=ps[:]
                            )
                        else:
                            nc.scalar.copy(
                                out=qTs[dm][half][:, qc * 512 : (qc + 1) * 512],
                                in_=ps[:],
                            )
        # v: out v[j*128+p, dl] ; stationary = xvT chunk, moving = wv
        for half in range(2):
            x_t = xpool.tile([128, 8, 1024], BF16, tag="x")
            for quarter in range(2):
                nc.sync.dma_start(
                    x_t[:, quarter * 4 : (quarter + 1) * 4, :],
                    xvT[
                        quarter * 512 : (quarter + 1) * 512,
                        half * 1024 : (half + 1) * 1024,
                    ].rearrange("(a p) q -> p a q", p=128),
                )
            for km in range(8):
                ps = pp.tile([128, 512], F32, tag="pp")
                for ki in range(8):
                    nc.tensor.matmul(
                        ps[:, 0:DL],
                        lhsT=x_t[:, ki, km * 128 : (km + 1) * 128],
                        rhs=wv_sb[:, ki, :],
                        start=(ki == 0),
                        stop=(ki == 7),
                    )
                nc.vector.tensor_copy(
                    vaugs[half * 8 + km][:, :, 64 : 64 + DH],
                    ps[:, 0:DL].rearrange("p (h d) -> p h d", h=HL),
                )

        # ---------------- attention + output projection ----------------
        outTs = {}

        def emit_wo_group(qp, outT_sb, mq, oc):
            qg = qp * 4 + mq
            ps = pp.tile([128, 512], F32, tag="pp", name=f"wops{qp}_{mq}_{oc}")
            for kc in range(2):
                nc.tensor.matmul(
                    ps[:],
                    lhsT=outT_sb[:, kc, mq * 128 : (mq + 1) * 128],
                    rhs=wo_sb[:, kc, oc * 512 : (oc + 1) * 512],
                    start=(kc == 0),
                    stop=(kc == 1),
                )
            o_sb = ospool.tile([128, 512], F32, tag="o", name=f"wo_o{qp}_{mq}_{oc}")
            nc.vector.tensor_scalar_mul(o_sb[:], ps[:], mask_sb[:, qg : qg + 1])
            nc.sync.dma_start(
                out_part[qg * 128 : (qg + 1) * 128, oc * 512 : (oc + 1) * 512],
                o_sb[:],
            )

        for qp in range(4):  # 512-query blocks
            q0 = qp * 512
            outT_sb = opool.tile([128, 2, 512], BF16, tag="outT", name=f"outT{qp}")
            outTs[qp] = outT_sb
            for hp in range(2):  # head pairs (2*hp, 2*hp+1)
                acc = [
                    accp.tile([128, 512], F32, tag="acc", name=f"acc{qp}_{hp}_{i}")
                    for i in range(2)
                ]
                pending_wo = []
                for j in range(16):  # 128-key chunks
                    st = stp.tile([128, 1024], F32, tag="st")
                    for hi in range(2):  # even/odd head -> PE rows 0:64 / 64:128
                        r0 = hi * 64
                        nc.tensor.matmul(
                            st[:, hi * 512 : (hi + 1) * 512],
                            lhsT=kTs[hp][r0 : r0 + 64, j * 128 : (j + 1) * 128],
                            rhs=qTs[hp][q0 // 1024][
                                r0 : r0 + 64, (q0 % 1024) : (q0 % 1024) + 512
                            ],
                            start=True,
                            stop=True,
                        )
                    e_t = epool.tile([128, 1024], BF16, tag="e")
                    nc.scalar.activation(out=e_t[:], in_=st[:], func=AF.Exp, scale=0.125)
                    for hi in range(2):
                        h = 2 * hp + hi
                        nc.tensor.matmul(
                            acc[hi][:],
                            lhsT=vaugs[j][:, h, :],
                            rhs=e_t[:, hi * 512 : (hi + 1) * 512],
                            start=(j == 0),
                            stop=(j == 15),
                        )
                    if pending_wo and j % 2 == 1:
                        emit_wo_group(*pending_wo.pop(0))
                for hi in range(2):
                    h = 2 * hp + hi
                    acc_sb = rpool.tile([128, 512], F32, tag="accsb")
                    nc.vector.tensor_copy(acc_sb[:], acc[hi][:])
                    r_sb = rpool.tile([1, 512], F32, tag="r")
                    nc.vector.reciprocal_approx_fast(out=r_sb[:], in_=acc_sb[0:1, :])
                    if qp == 3 and hp == 1:
                        # tail: broadcast on-chip (PE outer product), keeping the
                        # final W_o off the DMA round-trip latency
                        rb_ps = pp.tile(
                            [64, 512], F32, tag="pp", name=f"rbps{qp}_{hp}_{hi}"
                        )
                        nc.tensor.matmul(
                            rb_ps[:], lhsT=ones_sb[:], rhs=r_sb[:], start=True, stop=True
                        )
                        nc.vector.tensor_mul(
                            outT_sb[hi * 64 : (hi + 1) * 64, hp, :],
                            acc_sb[64 : 64 + DH, :],
                            rb_ps[:],
                        )
                    else:
                        row = qp * 4 + hp * 2 + hi
                        nc.sync.dma_start(r_dram[row : row + 1, :], r_sb[:])
                        rb_sb = rpool.tile([128, 512], F32, tag="rb")
                        src = r_dram[row : row + 1, :]
                        nc.sync.dma_start(
                            rb_sb[64:128, :],
                            bass.AP(
                                tensor=src.tensor,
                                offset=src.offset,
                                ap=[[0, 64]] + src.ap[1:],
                            ),
                        )
                        nc.vector.tensor_mul(
                            outT_sb[hi * 64 : (hi + 1) * 64, hp, :],
                            acc_sb[64 : 64 + DH, :],
                            rb_sb[64:128, :],
                        )
            for mq in range(4):
                for oc in range(2):
                    emit_wo_group(qp, outT_sb, mq, oc)


    nc.compile()
    return nc


def _get_program():
    global _PROGRAM
    if _PROGRAM is None:
        _PROGRAM = build_program()
    return _PROGRAM


def make_in_maps(Q, K, V, mask, W_q, W_k, W_v, W_o):
    bf = ml_dtypes.bfloat16
    Q, K, V = (np.asarray(a, np.float32) for a in (Q, K, V))
    W_q, W_k, W_v, W_o = (np.asarray(a, np.float32) for a in (W_q, W_k, W_v, W_o))
    mask = np.asarray(mask)
    in_maps = []
    for core in range(NCORES):
        b, hg = core // 4, core % 4
        c0 = hg * DL
        in_maps.append(
            {
                "xqT": np.ascontiguousarray(Q[b].T).astype(bf),
                "xkT": np.ascontiguousarray(K[b].T).astype(bf),
                "xvT": np.ascontiguousarray(V[b].T).astype(bf),
                "wq": np.ascontiguousarray(W_q[c0 : c0 + DL, :].T).astype(bf),
                "wk": np.ascontiguousarray(W_k[c0 : c0 + DL, :].T).astype(bf),
                "wv": np.ascontiguousarray(W_v[c0 : c0 + DL, :].T).astype(bf),
                "wo": np.ascontiguousarray(W_o[:, c0 : c0 + DL].T).astype(bf),
                "maskf": np.ascontiguousarray(
                    mask[b].reshape(SEQ // 128, 128).T
                ).astype(np.float32),
            }
        )
    return in_maps


def gather(results):
    out = np.zeros((B, SEQ, D), np.float32)
    for core in range(NCORES):
        out[core // 4] += results[core]["out_part"]
    return out


def kernel(Q, K, V, mask, W_q, W_k, W_v, W_o):
    from concourse.bass_utils import run_bass_kernel_spmd

    nc = _get_program()
    in_maps = make_in_maps(Q, K, V, mask, W_q, W_k, W_v, W_o)
    res = run_bass_kernel_spmd(nc, in_maps, list(range(NCORES))).results
    return gather(res)


# revision 17
# speedup vs baseline: 1.1742x; 1.1742x over previous
"""MultiHeadCrossAttention kernel for 8 trn2 NeuronCores.

Reference computation (fp32, per batch b):
    q = Q[b] @ W_q.T ; k = K[b] @ W_k.T ; v = V[b] @ W_v.T      (heads on columns)
    per head h: S = (q_h @ k_h.T) / 8 ; E = exp(S); A = E / E.sum(-1)
    out[b] = concat_h(A @ v_h) @ W_o.T ; rows with mask==0 zeroed

Sharding: 8 cores = (batch b in {0,1}) x (head-group hg in {0..3}, 4 heads each).
Each core computes a partial output  out_part[b] = concat(heads hg) @ W_o[:, cols].T
and the host sums the 4 partials per batch.

Per-core kernel layout (all matmul operands bf16, fp32 PSUM accumulation):
  - xqT/xkT/xvT: host-transposed [1024(in), 2048(seq)] so the contraction dim
    (model dim) lands on SBUF partitions.
  - qT/kT stored as [128, 2, 2048]: partition = local-head-dim % 128. Head pair
    hp occupies chunk hp; even head rows 0:64, odd head rows 64:128 - so the two
    scores matmuls of a pair use disjoint PE row groups and run concurrently.
  - Scores are computed transposed, ST[kpos, q], per 128-kpos chunk j; exp runs
    on ScalarE over a 2-bank [128, 1024] PSUM region (even|odd head halves) with
    the 1/8 scale fused.
  - PV: acc[128, 512] += [ones | 0*63 | v_h].T @ E  accumulated over j. Row 0 is
    the softmax denominator; rows 64:128 the unnormalized output ([d, q]).
  - Normalize: copy acc to SBUF (frees the PSUM bank fast), approx reciprocal
    of row 0, broadcast across partitions via a DRAM bounce + step-0 read (or a
    PE outer-product for the final pair, keeping the last W_o off the DMA
    latency), one tensor_mul into the W_o lhsT layout (bf16).
  - W_o: final[q,:] accumulated over the 256 local dims; the mask (per-q 0/1)
    is applied by the PSUM->SBUF tensor_scalar multiply before the output DMA.
"""

import numpy as np
import ml_dtypes

import concourse.bass as bass
import concourse.bacc as bacc
import concourse.mybir as mybir
import concourse.tile as tile
from contextlib import ExitStack

F32 = mybir.dt.float32
BF16 = mybir.dt.bfloat16
AF = mybir.ActivationFunctionType

B = 2
SEQ = 2048          # Sq == Sk
D = 1024            # model dim
DL = 256            # local head dims per core (4 heads x 64)
HL = 4              # local heads
DH = 64             # head dim
NCORES = 8

_PROGRAM = None


def build_program():
    nc = bacc.Bacc("TRN2", target_bir_lowering=False)

    xqT = nc.declare_dram_parameter("xqT", [D, SEQ], BF16, isOutput=False)
    xkT = nc.declare_dram_parameter("xkT", [D, SEQ], BF16, isOutput=False)
    xvT = nc.declare_dram_parameter("xvT", [D, SEQ], BF16, isOutput=False)
    wq = nc.declare_dram_parameter("wq", [D, DL], BF16, isOutput=False)
    wk = nc.declare_dram_parameter("wk", [D, DL], BF16, isOutput=False)
    wv = nc.declare_dram_parameter("wv", [D, DL], BF16, isOutput=False)
    wo = nc.declare_dram_parameter("wo", [DL, D], BF16, isOutput=False)
    maskf = nc.declare_dram_parameter("maskf", [128, SEQ // 128], F32, isOutput=False)
    out_part = nc.declare_dram_parameter("out_part", [SEQ, D], F32, isOutput=True)

    r_dram = nc.dram_tensor("r_bounce", [16, 512], F32)  # recip bounce rows


    with tile.TileContext(nc) as tc, ExitStack() as ctx:
        const = ctx.enter_context(tc.tile_pool(name="const", bufs=1))
        proj = ctx.enter_context(tc.tile_pool(name="proj", bufs=1))
        xpool = ctx.enter_context(tc.tile_pool(name="xpool", bufs=4))
        epool = ctx.enter_context(tc.tile_pool(name="epool", bufs=6))
        opool = ctx.enter_context(tc.tile_pool(name="opool", bufs=2))
        ospool = ctx.enter_context(tc.tile_pool(name="ospool", bufs=4))
        rpool = ctx.enter_context(tc.tile_pool(name="rpool", bufs=4))
        pp = ctx.enter_context(tc.tile_pool(name="pp", bufs=2, space="PSUM"))
        stp = ctx.enter_context(tc.tile_pool(name="stp", bufs=2, space="PSUM"))
        accp = ctx.enter_context(tc.tile_pool(name="accp", bufs=2, space="PSUM"))

        # ---------------- constants ----------------
        wq_sb = const.tile([128, 8, DL], BF16)
        wk_sb = const.tile([128, 8, DL], BF16)
        wv_sb = const.tile([128, 8, DL], BF16)
        wo_sb = const.tile([128, 2, D], BF16)
        mask_sb = const.tile([128, SEQ // 128], F32)
        ones_sb = const.tile([1, 64], F32)
        nc.vector.memset(ones_sb[:], 1.0)
        nc.scalar.dma_start(wq_sb[:], wq[:].rearrange("(a p) d -> p a d", p=128))
        nc.sync.dma_start(wk_sb[:], wk[:].rearrange("(a p) d -> p a d", p=128))
        nc.scalar.dma_start(wv_sb[:], wv[:].rearrange("(a p) d -> p a d", p=128))
        nc.scalar.dma_start(wo_sb[:], wo[:].rearrange("(a p) d -> p a d", p=128))
        nc.scalar.dma_start(mask_sb[:], maskf[:])

        kT0_sb = proj.tile([128, SEQ], BF16)
        kT1_sb = proj.tile([128, SEQ], BF16)
        kTs = (kT0_sb, kT1_sb)
        qTs = [
            [proj.tile([128, 1024], BF16, name=f"qT{dm}_{h}") for h in range(2)]
            for dm in range(2)
        ]
        vaugs = [
            proj.tile([128, HL, 128], BF16, name=f"vaug{j}") for j in range(16)
        ]
        for j in range(16):
            nc.vector.memset(vaugs[j][:], 0.0)
            nc.vector.memset(vaugs[j][:, :, 0:1], 1.0)

        # ---------------- projections ----------------
        # order: k fully, then q fully, then v (per-j vaug tiles let PV start
        # as soon as the first v chunks land)
        for w_sb, xT, which in ((wk_sb, xkT, "k"),):
            for half in range(2):
                x_t = xpool.tile([128, 8, 1024], BF16, tag="x")
                for quarter in range(2):
                    nc.sync.dma_start(
                        x_t[:, quarter * 4 : (quarter + 1) * 4, :],
                        xT[
                            quarter * 512 : (quarter + 1) * 512,
                            half * 1024 : (half + 1) * 1024,
                        ].rearrange("(a p) q -> p a q", p=128),
                    )
                for dm in range(2):
                    for qc in range(2):
                        ps = pp.tile([128, 512], F32, tag="pp")
                        for ki in range(8):
                            nc.tensor.matmul(
                                ps[:],
                                lhsT=w_sb[:, ki, dm * 128 : (dm + 1) * 128],
                                rhs=x_t[:, ki, qc * 512 : (qc + 1) * 512],
                                start=(ki == 0),
                                stop=(ki == 7),
                            )
                        col = half * 1024 + qc * 512
                        if which == "k":
                            nc.scalar.copy(
                                out=kTs[dm][:, col : col + 512], in_=ps[:]
                            )
                        else:
                            nc.scalar.copy(
                                out=qTs[dm][half][:, qc * 512 : (qc + 1) * 512],
                                in_=ps[:],
                            )
        for w_sb, xT, which in ((wq_sb, xqT, "q"),):
            for half in range(2):
                x_t = xpool.tile([128, 8, 1024], BF16, tag="x")
                for quarter in range(2):
                    nc.sync.dma_start(
                        x_t[:, quarter * 4 : (quarter + 1) * 4, :],
                        xT[
                            quarter * 512 : (quarter + 1) * 512,
                            half * 1024 : (half + 1) * 1024,
                        ].rearrange("(a p) q -> p a q", p=128),
                    )
                for dm in range(2):
                    for qc in range(2):
                        ps = pp.tile([128, 512], F32, tag="pp")
                        for ki in range(8):
                            nc.tensor.matmul(
                                ps[:],
                                lhsT=w_sb[:, ki, dm * 128 : (dm + 1) * 128],
                                rhs=x_t[:, ki, qc * 512 : (qc + 1) * 512],
                                start=(ki == 0),
                                stop=(ki == 7),
                            )
                        col = half * 1024 + qc * 512
                        if which == "k":
                            nc.scalar.copy(
                                out=kTs[dm][:, col : col + 512], in_=ps[:]
                            )
                        else:
                            nc.scalar.copy(
                                out=qTs[dm][half][:, qc * 512 : (qc + 1) * 512],
                                in_=ps[:],
                            )
        # v: out v[j*128+p, dl] ; stationary = xvT chunk, moving = wv
        for half in range(2):
            x_t = xpool.tile([128, 8, 1024], BF16, tag="x")
            for quarter in range(2):
                nc.sync.dma_start(
                    x_t[:, quarter * 4 : (quarter + 1) * 4, :],
                    xvT[
                        quarter * 512 : (quarter + 1) * 512,
                        half * 1024 : (half + 1) * 1024,
                    ].rearrange("(a p) q -> p a q", p=128),
                )
            for km in range(8):
                ps = pp.tile([128, 512], F32, tag="pp")
                for ki in range(8):
                    nc.tensor.matmul(
                        ps[:, 0:DL],
                        lhsT=x_t[:, ki, km * 128 : (km + 1) * 128],
                        rhs=wv_sb[:, ki, :],
                        start=(ki == 0),
                        stop=(ki == 7),
                    )
                nc.vector.tensor_copy(
                    vaugs[half * 8 + km][:, :, 64 : 64 + DH],
                    ps[:, 0:DL].rearrange("p (h d) -> p h d", h=HL),
                )

        # ---------------- attention + output projection ----------------
        outTs = {}

        def emit_wo_group(qp, outT_sb, mq, oc):
            qg = qp * 4 + mq
            ps = pp.tile([128, 512], F32, tag="pp", name=f"wops{qp}_{mq}_{oc}")
            for kc in range(2):
                nc.tensor.matmul(
                    ps[:],
                    lhsT=outT_sb[:, kc, mq * 128 : (mq + 1) * 128],
                    rhs=wo_sb[:, kc, oc * 512 : (oc + 1) * 512],
                    start=(kc == 0),
                    stop=(kc == 1),
                )
            o_sb = ospool.tile([128, 512], F32, tag="o", name=f"wo_o{qp}_{mq}_{oc}")
            nc.vector.tensor_scalar_mul(o_sb[:], ps[:], mask_sb[:, qg : qg + 1])
            nc.sync.dma_start(
                out_part[qg * 128 : (qg + 1) * 128, oc * 512 : (oc + 1) * 512],
                o_sb[:],
            )

        for qp in range(4):  # 512-query blocks
            q0 = qp * 512
            outT_sb = opool.tile([128, 2, 512], BF16, tag="outT", name=f"outT{qp}")
            outTs[qp] = outT_sb
            for hp in range(2):  # head pairs (2*hp, 2*hp+1)
                acc = [
                    accp.tile([128, 512], F32, tag="acc", name=f"acc{qp}_{hp}_{i}")
                    for i in range(2)
                ]
                pending_wo = []
                for j in range(16):  # 128-key chunks
                    st = stp.tile([128, 1024], F32, tag="st")
                    for hi in range(2):  # even/odd head -> PE rows 0:64 / 64:128
                        r0 = hi * 64
                        nc.tensor.matmul(
                            st[:, hi * 512 : (hi + 1) * 512],
                            lhsT=kTs[hp][r0 : r0 + 64, j * 128 : (j + 1) * 128],
                            rhs=qTs[hp][q0 // 1024][
                                r0 : r0 + 64, (q0 % 1024) : (q0 % 1024) + 512
                            ],
                            start=True,
                            stop=True,
                        )
                    e_t = epool.tile([128, 1024], BF16, tag="e")
                    nc.scalar.activation(out=e_t[:], in_=st[:], func=AF.Exp, scale=0.125)
                    for hi in range(2):
                        h = 2 * hp + hi
                        nc.tensor.matmul(
                            acc[hi][:],
                            lhsT=vaugs[j][:, h, :],
                            rhs=e_t[:, hi * 512 : (hi + 1) * 512],
                            start=(j == 0),
                            stop=(j == 15),
                        )
                    if pending_wo and j % 2 == 1:
                        emit_wo_group(*pending_wo.pop(0))
                for hi in range(2):
                    h = 2 * hp + hi
                    acc_sb = rpool.tile([128, 512], F32, tag="accsb")
                    nc.vector.tensor_copy(acc_sb[:], acc[hi][:])
                    r_sb = rpool.tile([1, 512], F32, tag="r")
                    nc.vector.reciprocal_approx_fast(out=r_sb[:], in_=acc_sb[0:1, :])
                    if qp == 3 and hp == 1:
                        # tail: broadcast on-chip (PE outer product), keeping the
                        # final W_o off the DMA round-trip latency
                        rb_ps = pp.tile(
                            [64, 512], F32, tag="pp", name=f"rbps{qp}_{hp}_{hi}"
                        )
                        nc.tensor.matmul(
                            rb_ps[:], lhsT=ones_sb[:], rhs=r_sb[:], start=True, stop=True
                        )
                        nc.vector.tensor_mul(
                            outT_sb[hi * 64 : (hi + 1) * 64, hp, :],
                            acc_sb[64 : 64 + DH, :],
                            rb_ps[:],
                        )
                    else:
                        row = qp * 4 + hp * 2 + hi
                        nc.sync.dma_start(r_dram[row : row + 1, :], r_sb[:])
                        rb_sb = rpool.tile([128, 512], F32, tag="rb")
                        src = r_dram[row : row + 1, :]
                        nc.sync.dma_start(
                            rb_sb[64:128, :],
                            bass.AP(
                                tensor=src.tensor,
                                offset=src.offset,
                                ap=[[0, 64]] + src.ap[1:],
                            ),
                        )
                        nc.vector.tensor_mul(
                            outT_sb[hi * 64 : (hi + 1) * 64, hp, :],
                            acc_sb[64 : 64 + DH, :],
                            rb_sb[64:128, :],
                        )
            for mq in range(4):
                for oc in range(2):
                    emit_wo_group(qp, outT_sb, mq, oc)


    nc.compile()
    return nc


def _get_program():
    global _PROGRAM
    if _PROGRAM is None:
        _PROGRAM = build_program()
    return _PROGRAM


def make_in_maps(Q, K, V, mask, W_q, W_k, W_v, W_o):
    bf = ml_dtypes.bfloat16
    Q, K, V = (np.asarray(a, np.float32) for a in (Q, K, V))
    W_q, W_k, W_v, W_o = (np.asarray(a, np.float32) for a in (W_q, W_k, W_v, W_o))
    mask = np.asarray(mask)
    in_maps = []
    for core in range(NCORES):
        b, hg = core // 4, core % 4
        c0 = hg * DL
        in_maps.append(
            {
                "xqT": np.ascontiguousarray(Q[b].T).astype(bf),
                "xkT": np.ascontiguousarray(K[b].T).astype(bf),
                "xvT": np.ascontiguousarray(V[b].T).astype(bf),
                "wq": np.ascontiguousarray(W_q[c0 : c0 + DL, :].T).astype(bf),
                "wk": np.ascontiguousarray(W_k[c0 : c0 + DL, :].T).astype(bf),
                "wv": np.ascontiguousarray(W_v[c0 : c0 + DL, :].T).astype(bf),
                "wo": np.ascontiguousarray(W_o[:, c0 : c0 + DL].T).astype(bf),
                "maskf": np.ascontiguousarray(
                    mask[b].reshape(SEQ // 128, 128).T
                ).astype(np.float32),
            }
        )
    return in_maps


def gather(results):
    out = np.zeros((B, SEQ, D), np.float32)
    for core in range(NCORES):
        out[core // 4] += results[core]["out_part"]
    return out


def kernel(Q, K, V, mask, W_q, W_k, W_v, W_o):
    from concourse.bass_utils import run_bass_kernel_spmd

    nc = _get_program()
    in_maps = make_in_maps(Q, K, V, mask, W_q, W_k, W_v, W_o)
    res = run_bass_kernel_spmd(nc, in_maps, list(range(NCORES))).results
    return gather(res)
